# revision 56
# baseline (speedup 1.0000x reference)
"""Multi-head self-attention + residual + LayerNorm on 8 Trainium2 NeuronCores.

Problem: B=4, S=2048, D=1024, H=16, d_k=64, fp32.

Sharding: token-parallel, zero collectives. Core c owns batch b=c//2 and a
1024-query-token half of it. Each core recomputes K/V projections for its full
batch (25% redundant FLOPs — cheaper than any collective at this size). The
per-core x^T is rotated on the host so every core's own query tokens sit in
columns 0..1023, keeping the device program identical across cores (softmax
and attn@V are permutation-invariant over the key axis, so rotating K/V rows
together is harmless).

v3 layout (all matmul operands bf16, psum/LN arithmetic f32):
 - x^T stays resident in SBUF all kernel (one 4MB load), so projection
   groups are free-floating: a minimal head computes only what the first
   attention blocks need (kT-ft0/ft1, Q-hp0, all of V for head-group 0);
   every other projection (rest of K0/Q0, all of hg1's K/V/Q) is woven
   into the attention windows' engine slack in deadline order, one
   8-matmul psum group at a time (b0_list / b1_list).
 - Phase B0 (attention for hg0): per (qg, j, kt): two K=64 score matmuls
   packed via tile_position row groups, one Exp on ScalarE (1/sqrt(dk)
   folded into scale, no max-shift: scores ~N(0,1)), then attn@V as eight
   [q=128, 65] matmuls (lhsT = e[k,q], rhs = v_aug[k, dk+1], 65th column
   ones -> row sums land in column 64 for free), plus ~2.5 woven
   projection matmuls per iteration.
 - Normalize: reciprocal of psum column 64 gives per-query (per-partition)
   denominators; one tensor_scalar multiply per q-tile — no partition
   broadcast needed. PE transposes (against a bf16 identity) flip the
   normalized [q, dk] tiles into the o-proj lhsT layout [dk, q].
 - Phase B1 (attention for hg1): same loop; o-proj+residual+LayerNorm
   c_blocks for the finished query group are woven in one matmul per kt.
   Each block's last iteration pre-emits the next block's first two
   scores+exps ahead of the attnV (whose exp-wait overflows PE's 4-deep
   wait queue), so ScalarE never idles across block boundaries.
 - LayerNorm: bn_stats/bn_aggr on DVE; rstd = exp(-0.5*ln(var+eps)) on
   ScalarE — Ln and Exp share one activation table set (pinned at build
   time), so the attention Exp table is loaded exactly once and never
   thrashes (Sqrt's table does not contain Exp).
"""

import numpy as np

import concourse.mybir as mybir
import concourse.tile as tile
from concourse import bacc
from concourse import bass_utils

F32 = mybir.dt.float32
F32R = mybir.dt.float32r
BF16 = mybir.dt.bfloat16

B, S, D, H, DK = 4, 2048, 1024, 16, 64
N_CORES = 8
TOK = (B * S) // N_CORES            # 1024 query tokens per core
NKT = S // 128                      # 16 k-tiles per batch
NFT = D // 128                      # 8 feature tiles
NTG = S // 512                      # 4 token groups per batch
EPS = 1e-5

_CACHE = {}


def build(apply_gb: bool, apply_bias: bool):
    nc = bacc.Bacc("TRN2", target_bir_lowering=False, debug=False,
                   num_devices=N_CORES)
    # The kernel's only ScalarE functions are Exp (attention) and Ln (the
    # LayerNorm rstd = exp(-0.5*ln(var+eps)) path). Both live in the
    # natural_log_exp_and_others table, but the greedy table chooser maps
    # each func to the first table containing it (exp_and_others /
    # natural_log), which forces a 1283ns table reload around every Ln.
    # Emptying the competing sets in the cached tables dict (positions —
    # and thus act_func_set ids — preserved) pins every activation to the
    # shared table, so it is loaded exactly once. Runtime table contents
    # come from the compiler's own act_info.json and are unaffected.
    from concourse.hw_specs import get_activation_tables
    tabs = get_activation_tables(nc.m.arch)
    for name, s in tabs.items():
        if name != "natural_log_exp_and_others":
            s.discard(mybir.ActivationFunctionType.Exp)
            s.discard(mybir.ActivationFunctionType.Ln)
    xT_d = nc.dram_tensor("xT", [D, S], BF16, kind="ExternalInput")
    xmy_d = nc.dram_tensor("xmy", [TOK, D], BF16, kind="ExternalInput")
    wqT_d = nc.dram_tensor("wqT", [D, D], BF16, kind="ExternalInput")
    wkT_d = nc.dram_tensor("wkT", [D, D], BF16, kind="ExternalInput")
    wvT_d = nc.dram_tensor("wvT", [D, D], BF16, kind="ExternalInput")
    woT_d = nc.dram_tensor("woT", [D, D], BF16, kind="ExternalInput")
    ident_d = nc.dram_tensor("ident", [128, 128], BF16, kind="ExternalInput")
    bo_d = nc.dram_tensor("bo", [1, D], F32, kind="ExternalInput")
    gb_d = nc.dram_tensor("gb", [2, D], F32, kind="ExternalInput")
    y_d = nc.dram_tensor("y", [TOK, D], F32, kind="ExternalOutput")

    with tile.TileContext(nc) as tc:
        with (
            tc.tile_pool(name="big", bufs=1) as big,
            tc.tile_pool(name="vpool", bufs=2) as vpool,
            tc.tile_pool(name="xr", bufs=3) as xr,
            tc.tile_pool(name="ev", bufs=5) as ev,
            tc.tile_pool(name="on", bufs=2) as onp,
            tc.tile_pool(name="small", bufs=1) as small,
            tc.tile_pool(name="ln", bufs=2) as lnp,
            tc.tile_pool(name="ps_sc", bufs=2, space="PSUM") as ps_sc,
            tc.tile_pool(name="ps_o", bufs=2, space="PSUM") as ps_o,
            tc.tile_pool(name="ps_mm", bufs=2, space="PSUM") as ps_mm,
        ):
            # persistent operand tiles: both head groups side by side.
            # head pair hp = hg*4 + j lives at index hp; head 2hp in rows
            # 0:64, head 2hp+1 in rows 64:128.
            kT = big.tile([128, NFT, S], BF16, tag="kT")             # 4 MB
            qT = big.tile([128, NFT, TOK], BF16, tag="qT")           # 2 MB
            oT = big.tile([128, NFT, TOK], BF16, tag="oT")           # 2 MB
            wk = big.tile([128, NFT, D], BF16, tag="wk")             # 2 MB
            wv = big.tile([128, NFT, D], BF16, tag="wv")             # 2 MB
            wq = big.tile([128, NFT, D], BF16, tag="wq")             # 2 MB
            ident = big.tile([128, 128], BF16, tag="ident")

            def new_vaug():
                # [k-tile rows, kt, 8 heads x (dk | 1)] with ones in col dk
                va = vpool.tile([128, NKT, 8 * (DK + 1)], BF16, tag="vaug")
                nc.vector.memset(
                    va[:].rearrange("p t (h c) -> p t h c", h=8)[:, :, :, DK:DK + 1],
                    1.0,
                )
                return va

            vaug = [None, None]
            vaug[0] = new_vaug()

            def emit_scores_exp(hp, qg, kt):
                sc = ps_sc.tile([128, 1024], F32, tag="sc")
                nc.tensor.matmul(
                    sc[:, 0:512],
                    kT[0:64, hp, kt * 128:(kt + 1) * 128],
                    qT[0:64, hp, qg * 512:(qg + 1) * 512],
                    start=True, stop=True, tile_position=(0, 0),
                )
                nc.tensor.matmul(
                    sc[:, 512:1024],
                    kT[64:128, hp, kt * 128:(kt + 1) * 128],
                    qT[64:128, hp, qg * 512:(qg + 1) * 512],
                    start=True, stop=True, tile_position=(64, 0),
                )
                e_ab = ev.tile([128, 1024], BF16, tag="exp")
                nc.scalar.activation(
                    out=e_ab[:], in_=sc[:],
                    func=mybir.ActivationFunctionType.Exp,
                    scale=0.125,
                )
                return e_ab

            def emit_attnv(hg, j, kt, e_ab, o_psA, o_psB):
                for hh, o_ps in ((0, o_psA), (1, o_psB)):
                    va = vaug[hg][:, kt, :]
                    for qt in range(4):
                        # one accumulation group per psum bank: start marks
                        # the whole 2KB zero-region pending, so only the
                        # bank's first matmul starts and only its last
                        # stops; the other q-tiles' first writes land on
                        # pending-zero bytes (fresh write)
                        nc.tensor.matmul(
                            o_ps[:, qt * 128:qt * 128 + DK + 1],
                            e_ab[:, hh * 512 + qt * 128:
                                 hh * 512 + (qt + 1) * 128],
                            va[:, (2 * j + hh) * (DK + 1):
                               (2 * j + hh + 1) * (DK + 1)],
                            start=(kt == 0 and qt == 0),
                            stop=(kt == NKT - 1 and qt == 3),
                        )

            # x^T stays resident all kernel (loaded once), so projection
            # groups can be woven into any attention window in deadline
            # order instead of being tied to a streaming token-group sweep
            xall = big.tile([128, NFT, S], BF16, tag="xall")         # 4 MB

            # ---------- projection group emitters ----------
            # "k": (hgx, ft, tg)  K^T head-pair column block for one tg
            # "q": (hp, qgx)      Q^T head pair for one query group (tg==qg)
            # "v": (hgx, kt)      V row block -> v_aug[hgx]
            def groups_gen(worklist):
                for item in worklist:
                    kind = item[0]
                    ps = ps_mm.tile([128, 512], F32, tag="mm512")
                    if kind == "k":
                        _, hgx, ft, tg = item
                        for d in range(NFT):
                            nc.tensor.matmul(
                                ps[:],
                                wk[:, d, hgx * 512 + ft * 128:
                                   hgx * 512 + (ft + 1) * 128],
                                xall[:, d, tg * 512:(tg + 1) * 512],
                                start=(d == 0), stop=(d == NFT - 1),
                            )
                            yield
                        nc.vector.tensor_copy(
                            kT[:, hgx * 4 + ft, tg * 512:(tg + 1) * 512], ps[:])
                    elif kind == "q":
                        _, hp, qgx = item
                        for d in range(NFT):
                            nc.tensor.matmul(
                                ps[:], wq[:, d, hp * 128:(hp + 1) * 128],
                                xall[:, d, qgx * 512:(qgx + 1) * 512],
                                start=(d == 0), stop=(d == NFT - 1),
                            )
                            yield
                        nc.vector.tensor_copy(
                            qT[:, hp, qgx * 512:(qgx + 1) * 512], ps[:])
                    else:
                        _, hgx, kt = item
                        for d in range(NFT):
                            nc.tensor.matmul(
                                ps[:], xall[:, d, kt * 128:(kt + 1) * 128],
                                wv[:, d, hgx * 512:(hgx + 1) * 512],
                                start=(d == 0), stop=(d == NFT - 1),
                            )
                            yield
                        nc.vector.tensor_copy(
                            out=vaug[hgx][:, kt, :]
                            .rearrange("p (h c) -> p h c", h=8)[:, :, 0:DK],
                            in_=ps[:].rearrange("p (h c) -> p h c", h=8),
                        )

            def emit_groups(worklist):
                for _ in groups_gen(worklist):
                    pass

            # ---------- Phase A: minimal head before attention starts ----
            # B0's first two j-blocks need kT-ft0/ft1, Q-hp0 and all of V0;
            # every other projection is woven into the attention windows
            # DMA queue order = first-use order: x tg0 + the K columns the
            # head needs, then the remaining x groups ahead of the bulkier
            # weight halves (the head reads every token group early, but
            # only wk-lo/wv-lo/wq-hp0 before its last group)
            for d in range(NFT):
                # all four token groups of this d-tile together with the
                # head's K columns: the head's K-ft0 groups consume x
                # tg-major at ~1.7us per group, so every tg must land early
                nc.sync.dma_start(
                    xall[:, d, 0:512], xT_d.ap()[d * 128:(d + 1) * 128, 0:512])
                nc.sync.dma_start(wk[:, d, 0:256],
                                  wkT_d.ap()[d * 128:(d + 1) * 128, 0:256])
                nc.sync.dma_start(
                    xall[:, d, 512:1024],
                    xT_d.ap()[d * 128:(d + 1) * 128, 512:1024])
            for tg in range(2, NTG):
                for d in range(NFT):
                    nc.sync.dma_start(
                        xall[:, d, tg * 512:(tg + 1) * 512],
                        xT_d.ap()[d * 128:(d + 1) * 128, tg * 512:(tg + 1) * 512],
                    )
            for d in range(NFT):
                nc.sync.dma_start(wv[:, d, 0:512],
                                  wvT_d.ap()[d * 128:(d + 1) * 128, 0:512])
            for d in range(NFT):
                nc.sync.dma_start(wq[:, d, 0:128],
                                  wqT_d.ap()[d * 128:(d + 1) * 128, 0:128])
            for d in range(NFT):
                nc.sync.dma_start(wk[:, d, 256:1024],
                                  wkT_d.ap()[d * 128:(d + 1) * 128, 256:1024])
            for d in range(NFT):
                nc.sync.dma_start(wq[:, d, 128:1024],
                                  wqT_d.ap()[d * 128:(d + 1) * 128, 128:1024])
            nc.sync.dma_start(ident[:], ident_d.ap())
            for d in range(NFT):
                nc.sync.dma_start(wv[:, d, 512:1024],
                                  wvT_d.ap()[d * 128:(d + 1) * 128, 512:1024])

            vaug[1] = new_vaug()
            emit_groups([("k", 0, 0, tg) for tg in range(NTG)]
                        + [("k", 0, 1, tg) for tg in range(NTG)]
                        + [("q", 0, 0)]
                        + [("v", 0, kt) for kt in range(NKT)])

            # deadline-ordered weave lists. B0 consumes them at ~2.5/kt
            # (pull p lands near iteration p/2.5); B1-qg0 at 2/kt. Each
            # entry is one 8-matmul psum group.
            b0_list = ([("q", 1, 0)]                       # j1 queries
                       + [("k", 0, 2, tg) for tg in range(NTG)]   # j2 keys
                       + [("q", 2, 0)]
                       + [("k", 0, 3, tg) for tg in range(NTG)]   # j3 keys
                       + [("q", 3, 0)]
                       + [("q", hp, 1) for hp in range(4)]  # B0-qg1 queries
                       + [("k", 1, 0, tg) for tg in range(NTG)]   # B1-j0 keys
                       + [("q", 4, 0)]                      # B1-j0 queries
                       + [("v", 1, kt) for kt in range(NKT)]
                       + [("q", 5, 0)])
            b1_list = ([("k", 1, 1, tg) for tg in range(NTG)]
                       + [("k", 1, 2, 0), ("k", 1, 2, 1), ("q", 6, 0),
                          ("k", 1, 2, 2), ("k", 1, 2, 3), ("q", 7, 0)]
                       + [("k", 1, 3, tg) for tg in range(NTG)]
                       + [("q", hp, 1) for hp in range(4, 8)])

            woT_box = [None]

            # ---------- c_block: o-proj + residual + LayerNorm ----------
            if apply_bias:
                bo_bc = small.tile([128, D], F32, tag="bobc")
                nc.sync.dma_start(bo_bc[:],
                                  bo_d.ap()[0:1, :].broadcast_to((128, D)))
            if apply_gb:
                g_bc = small.tile([128, D], F32, tag="gbc")
                b_bc = small.tile([128, D], F32, tag="bbc")
                nc.sync.dma_start(g_bc[:],
                                  gb_d.ap()[0:1, :].broadcast_to((128, D)))
                nc.sync.dma_start(b_bc[:],
                                  gb_d.ap()[1:2, :].broadcast_to((128, D)))

            ys_tags = ("wv", "vaug", "xall")
            eps_t = small.tile([128, 1], F32, tag="eps")
            nc.vector.memset(eps_t[:], EPS)

            def c_block_gen(tt, alt=False):
                """o-proj (16 matmul pulls) then residual+LN+store.

                alt=True draws the psum from the scores pool — free once
                attention has ended — so consecutive tail blocks pipeline
                instead of serializing on ps_mm's two banks."""
                woT = woT_box[0]
                x_t = xr.tile([128, D], BF16, tag="xres")
                nc.sync.dma_start(x_t[:], xmy_d.ap()[tt * 128:(tt + 1) * 128, :])
                if alt:
                    big_ps = ps_sc.tile([128, 1024], F32, tag="sc")
                    pss = [big_ps[:, 0:512], big_ps[:, 512:1024]]
                else:
                    ps_e0 = ps_mm.tile([128, 512], F32, tag="mm512")
                    ps_e1 = ps_mm.tile([128, 512], F32, tag="mm512")
                    pss = [ps_e0[:], ps_e1[:]]
                for eh in range(2):
                    for ft in range(NFT):
                        nc.tensor.matmul(
                            pss[eh], oT[:, ft, tt * 128:(tt + 1) * 128],
                            woT[:, ft, eh * 512:(eh + 1) * 512],
                            start=(ft == 0), stop=(ft == NFT - 1),
                        )
                        yield
                ys_tag = ys_tags[tt % len(ys_tags)]
                if ys_tag == "vaug":
                    y_sb = vpool.tile([128, D], F32, tag=ys_tag)
                else:
                    y_sb = big.tile([128, D], F32, tag=ys_tag)
                for eh in range(2):
                    nc.vector.tensor_add(
                        y_sb[:, eh * 512:(eh + 1) * 512],
                        pss[eh], x_t[:, eh * 512:(eh + 1) * 512],
                    )
                if apply_bias:
                    nc.vector.tensor_add(y_sb[:], y_sb[:], bo_bc[:])
                stats = lnp.tile([128, 2, nc.vector.BN_STATS_DIM], F32, tag="st")
                nc.vector.bn_stats(stats[:, 0, :], y_sb[:, 0:512])
                nc.vector.bn_stats(stats[:, 1, :], y_sb[:, 512:1024])
                mv = lnp.tile([128, nc.vector.BN_AGGR_DIM], F32, tag="mv")
                nc.vector.bn_aggr(mv[:], stats[:])
                lnv = lnp.tile([128, 1], F32, tag="lnv")
                rstd = lnp.tile([128, 1], F32, tag="rstd")
                # rstd = exp(-0.5*ln(var+eps)): Ln and Exp share one ScalarE
                # activation table set, so the attention Exp table never
                # reloads mid-kernel (Sqrt's table does not contain Exp)
                nc.scalar.activation(
                    out=lnv[:], in_=mv[:, 1:2],
                    func=mybir.ActivationFunctionType.Ln,
                    bias=eps_t[:], scale=1.0,
                )
                nc.scalar.activation(
                    out=rstd[:], in_=lnv[:],
                    func=mybir.ActivationFunctionType.Exp,
                    scale=-0.5,
                )
                nc.vector.tensor_scalar(
                    out=y_sb[:], in0=y_sb[:],
                    scalar1=mv[:, 0:1], scalar2=rstd[:],
                    op0=mybir.AluOpType.subtract, op1=mybir.AluOpType.mult,
                )
                if apply_gb:
                    nc.vector.tensor_mul(y_sb[:], y_sb[:], g_bc[:])
                    nc.vector.tensor_add(y_sb[:], y_sb[:], b_bc[:])
                nc.sync.dma_start(y_d.ap()[tt * 128:(tt + 1) * 128, :], y_sb[:])

            def pull(gen, n):
                if gen is None:
                    return None
                for _ in range(n):
                    try:
                        next(gen)
                    except StopIteration:
                        return None
                return gen

            def drain(gen):
                if gen is not None:
                    for _ in gen:
                        pass

            # ---------- Phase B: attention (hg0 then hg1) ----------
            blocks = [(hg, qg, j)
                      for hg in range(2) for qg in range(2) for j in range(4)]
            weave = groups_gen(b0_list)
            weave_n = (3, 2)        # ~2.5 pulls/kt through B0
            lead = {}
            for bi, (hg, qg, j) in enumerate(blocks):
                hp = hg * 4 + j
                if bi == 8:
                    # B1-qg0: late projection groups fill its ScalarE-bound
                    # slack (K1-ft1..3 land one j-block ahead of their use)
                    weave = groups_gen(b1_list)
                    # 40 pulls/block (vs 32): b1_list's late evictions
                    # (K1-ft3-tg1, q7) must land before bi10's kt4 lead
                    weave_n = (3, 2)
                elif bi >= 12:
                    # c_block for the query group finished one step ago:
                    # tts 0-3 here, the tail covers 4-7
                    weave = c_block_gen(qg * 4 + j - 4)
                    weave_n = (1, 1)
                o_psA = ps_o.tile([128, 512], F32, tag="o")
                o_psB = ps_o.tile([128, 512], F32, tag="o")
                lead_emitted = bi + 1 >= len(blocks)
                for kt in range(NKT):
                    e_ab = lead.pop((hp, qg, kt), None)
                    if e_ab is None:
                        e_ab = emit_scores_exp(hp, qg, kt)
                    if kt == NKT - 1 and not lead_emitted:
                        # pre-emit the next block's first two scores+exps
                        # BEFORE this attnV: attnV(kt15) waits on exp(kt15)
                        # and its 8 matmuls overflow PE's 4-deep wait queue,
                        # so anything after it stalls; emitting the lead
                        # scores first keeps ScalarE fed across the boundary
                        nhg, nqg, nj = blocks[bi + 1]
                        for ktl in range(2):
                            lead[(nhg * 4 + nj, nqg, ktl)] = \
                                emit_scores_exp(nhg * 4 + nj, nqg, ktl)
                        lead_emitted = True
                    emit_attnv(hg, j, kt, e_ab, o_psA, o_psB)
                    if kt == NKT - 1 and lead_emitted and bi + 1 < len(blocks):
                        # two more leads right after the last attnV: four
                        # queued exps (~4.2us) keep ScalarE busy through the
                        # epilogue chain (normalize -> transpose -> evict ->
                        # next block's o_ps rotation, ~3.4us) that delays
                        # the next block's own scores
                        nhg, nqg, nj = blocks[bi + 1]
                        for ktl in (2, 3, 4):
                            if (nhg * 4 + nj, nqg, ktl) not in lead:
                                lead[(nhg * 4 + nj, nqg, ktl)] = \
                                    emit_scores_exp(nhg * 4 + nj, nqg, ktl)
                    weave = pull(weave, weave_n[kt % 2])
                if not lead_emitted:
                    nhg, nqg, nj = blocks[bi + 1]
                    for ktl in range(2):
                        lead[(nhg * 4 + nj, nqg, ktl)] = \
                            emit_scores_exp(nhg * 4 + nj, nqg, ktl)
                if bi == 7 or bi >= 11:
                    # end of B0 / end of B1-qg0: finish weave leftovers;
                    # c_blocks: run the woven block's finalize
                    drain(weave)
                    weave = None
                if bi == 11:
                    # wk's readers (b1_list K groups) are all emitted: its
                    # slot now takes the o-proj weights, first-used half
                    # first; the c_blocks start ~15us later
                    woT_t = big.tile([128, NFT, D], BF16, tag="wk")
                    for d in range(NFT):
                        nc.sync.dma_start(
                            woT_t[:, d, 0:512],
                            woT_d.ap()[d * 128:(d + 1) * 128, 0:512])
                    for d in range(NFT):
                        nc.sync.dma_start(
                            woT_t[:, d, 512:1024],
                            woT_d.ap()[d * 128:(d + 1) * 128, 512:1024])
                    woT_box[0] = woT_t
                # normalize by the softmax sums (psum col 64 of each
                # q-tile), then PE-transpose into o-proj layout
                for hh, o_ps in ((0, o_psA), (1, o_psB)):
                    rec = onp.tile([128, 4], F32, tag="rec")
                    nc.vector.reciprocal(
                        rec[:],
                        o_ps[:].rearrange("p (q c) -> p q c", c=128)[:, :, DK],
                    )
                    o_nrm = onp.tile([128, 4, DK], BF16, tag="onrm")
                    for qt in range(4):
                        nc.vector.tensor_scalar(
                            out=o_nrm[:, qt, :],
                            in0=o_ps[:, qt * 128:qt * 128 + DK],
                            scalar1=rec[:, qt:qt + 1], scalar2=None,
                            op0=mybir.AluOpType.mult,
                        )
                    tr = ps_o.tile([128, 512], BF16, tag="o")
                    for qt in range(4):
                        nc.tensor.transpose(
                            tr[0:DK, qt * 128:(qt + 1) * 128],
                            o_nrm[:, qt, :], ident[:],
                        )
                    nc.vector.tensor_copy(
                        oT[hh * 64:(hh + 1) * 64, hp,
                           qg * 512:(qg + 1) * 512],
                        tr[0:DK, :],
                    )

            # tail: c_blocks for the last query group
            for tt in range(4, 8):
                drain(c_block_gen(tt, alt=bool(tt % 2)))

    nc.compile()
    return nc


def kernel(x, w_q, w_k, w_v, w_o, b_o, ln_g, ln_b):
    import ml_dtypes

    x = np.asarray(x, dtype=np.float32)
    w_q = np.asarray(w_q, dtype=np.float32)
    w_k = np.asarray(w_k, dtype=np.float32)
    w_v = np.asarray(w_v, dtype=np.float32)
    w_o = np.asarray(w_o, dtype=np.float32)
    b_o = np.asarray(b_o, dtype=np.float32)
    ln_g = np.asarray(ln_g, dtype=np.float32)
    ln_b = np.asarray(ln_b, dtype=np.float32)

    apply_gb = not (np.all(ln_g == 1.0) and np.all(ln_b == 0.0))
    apply_bias = bool(np.any(b_o != 0.0))
    key = (apply_gb, apply_bias)
    if key not in _CACHE:
        _CACHE[key] = build(apply_gb, apply_bias)
    nc = _CACHE[key]

    bf16 = ml_dtypes.bfloat16
    wqT = np.ascontiguousarray(w_q.T).astype(bf16)
    wkT = np.ascontiguousarray(w_k.T).astype(bf16)
    wvT = np.ascontiguousarray(w_v.T).astype(bf16)
    woT = np.ascontiguousarray(w_o.T).astype(bf16)
    ident = np.eye(128, dtype=np.float32).astype(bf16)
    gb = np.stack([ln_g, ln_b]).astype(np.float32)
    bo = np.ascontiguousarray(b_o.reshape(1, D))

    in_maps = []
    for c in range(N_CORES):
        b = c // 2
        half = c % 2
        xb = x[b]
        xT = np.ascontiguousarray(xb.T)
        if half == 1:
            xT = np.ascontiguousarray(np.roll(xT, -TOK, axis=1))
        xmy = np.ascontiguousarray(xb[half * TOK:(half + 1) * TOK]).astype(bf16)
        in_maps.append({
            "xT": xT.astype(bf16), "xmy": xmy,
            "wqT": wqT, "wkT": wkT, "wvT": wvT, "woT": woT,
            "ident": ident, "bo": bo, "gb": gb,
        })

    res = bass_utils.run_bass_kernel_spmd(nc, in_maps, core_ids=list(range(N_CORES)))
    y = np.stack([res.results[c]["y"] for c in range(N_CORES)])
    return y.reshape(B, S, D)


# revision 57
# speedup vs baseline: 1.0056x; 1.0056x over previous
"""Multi-head self-attention + residual + LayerNorm on 8 Trainium2 NeuronCores.

Problem: B=4, S=2048, D=1024, H=16, d_k=64, fp32.

Sharding: token-parallel, zero collectives. Core c owns batch b=c//2 and a
1024-query-token half of it. Each core recomputes K/V projections for its full
batch (25% redundant FLOPs — cheaper than any collective at this size). The
per-core x^T is rotated on the host so every core's own query tokens sit in
columns 0..1023, keeping the device program identical across cores (softmax
and attn@V are permutation-invariant over the key axis, so rotating K/V rows
together is harmless).

v3 layout (all matmul operands bf16, psum/LN arithmetic f32):
 - x^T stays resident in SBUF all kernel (one 4MB load), so projection
   groups are free-floating: a minimal head computes only what the first
   attention blocks need (kT-ft0/ft1, Q-hp0, all of V for head-group 0);
   every other projection (rest of K0/Q0, all of hg1's K/V/Q) is woven
   into the attention windows' engine slack in deadline order, one
   8-matmul psum group at a time (b0_list / b1_list).
 - Phase B0 (attention for hg0): per (qg, j, kt): two K=64 score matmuls
   packed via tile_position row groups, one Exp on ScalarE (1/sqrt(dk)
   folded into scale, no max-shift: scores ~N(0,1)), then attn@V as eight
   [q=128, 65] matmuls (lhsT = e[k,q], rhs = v_aug[k, dk+1], 65th column
   ones -> row sums land in column 64 for free), plus ~2.5 woven
   projection matmuls per iteration.
 - Normalize: reciprocal of psum column 64 gives per-query (per-partition)
   denominators; one tensor_scalar multiply per q-tile — no partition
   broadcast needed. PE transposes (against a bf16 identity) flip the
   normalized [q, dk] tiles into the o-proj lhsT layout [dk, q].
 - Phase B1 (attention for hg1): same loop; o-proj+residual+LayerNorm
   c_blocks for the finished query group are woven in one matmul per kt.
   Each block's last iteration pre-emits the next block's first two
   scores+exps ahead of the attnV (whose exp-wait overflows PE's 4-deep
   wait queue), so ScalarE never idles across block boundaries.
 - LayerNorm: bn_stats/bn_aggr on DVE; rstd = exp(-0.5*ln(var+eps)) on
   ScalarE — Ln and Exp share one activation table set (pinned at build
   time), so the attention Exp table is loaded exactly once and never
   thrashes (Sqrt's table does not contain Exp).
"""

import numpy as np

import concourse.mybir as mybir
import concourse.tile as tile
from concourse import bacc
from concourse import bass_utils

F32 = mybir.dt.float32
F32R = mybir.dt.float32r
BF16 = mybir.dt.bfloat16

B, S, D, H, DK = 4, 2048, 1024, 16, 64
N_CORES = 8
TOK = (B * S) // N_CORES            # 1024 query tokens per core
NKT = S // 128                      # 16 k-tiles per batch
NFT = D // 128                      # 8 feature tiles
NTG = S // 512                      # 4 token groups per batch
EPS = 1e-5

_CACHE = {}


def build(apply_gb: bool, apply_bias: bool):
    nc = bacc.Bacc("TRN2", target_bir_lowering=False, debug=False,
                   num_devices=N_CORES)
    # The kernel's only ScalarE functions are Exp (attention) and Ln (the
    # LayerNorm rstd = exp(-0.5*ln(var+eps)) path). Both live in the
    # natural_log_exp_and_others table, but the greedy table chooser maps
    # each func to the first table containing it (exp_and_others /
    # natural_log), which forces a 1283ns table reload around every Ln.
    # Emptying the competing sets in the cached tables dict (positions —
    # and thus act_func_set ids — preserved) pins every activation to the
    # shared table, so it is loaded exactly once. Runtime table contents
    # come from the compiler's own act_info.json and are unaffected.
    from concourse.hw_specs import get_activation_tables
    tabs = get_activation_tables(nc.m.arch)
    for name, s in tabs.items():
        if name != "natural_log_exp_and_others":
            s.discard(mybir.ActivationFunctionType.Exp)
            s.discard(mybir.ActivationFunctionType.Ln)
    xT_d = nc.dram_tensor("xT", [D, S], BF16, kind="ExternalInput")
    xmy_d = nc.dram_tensor("xmy", [TOK, D], BF16, kind="ExternalInput")
    wqT_d = nc.dram_tensor("wqT", [D, D], BF16, kind="ExternalInput")
    wkT_d = nc.dram_tensor("wkT", [D, D], BF16, kind="ExternalInput")
    wvT_d = nc.dram_tensor("wvT", [D, D], BF16, kind="ExternalInput")
    woT_d = nc.dram_tensor("woT", [D, D], BF16, kind="ExternalInput")
    ident_d = nc.dram_tensor("ident", [128, 128], BF16, kind="ExternalInput")
    bo_d = nc.dram_tensor("bo", [1, D], F32, kind="ExternalInput")
    gb_d = nc.dram_tensor("gb", [2, D], F32, kind="ExternalInput")
    y_d = nc.dram_tensor("y", [TOK, D], F32, kind="ExternalOutput")

    with tile.TileContext(nc) as tc:
        with (
            tc.tile_pool(name="big", bufs=1) as big,
            tc.tile_pool(name="vpool", bufs=2) as vpool,
            tc.tile_pool(name="xr", bufs=3) as xr,
            tc.tile_pool(name="ev", bufs=4) as ev,
            tc.tile_pool(name="on", bufs=2) as onp,
            tc.tile_pool(name="small", bufs=1) as small,
            tc.tile_pool(name="ln", bufs=2) as lnp,
            tc.tile_pool(name="ps_sc", bufs=2, space="PSUM") as ps_sc,
            tc.tile_pool(name="ps_o", bufs=2, space="PSUM") as ps_o,
            tc.tile_pool(name="ps_mm", bufs=2, space="PSUM") as ps_mm,
        ):
            # persistent operand tiles: both head groups side by side.
            # head pair hp = hg*4 + j lives at index hp; head 2hp in rows
            # 0:64, head 2hp+1 in rows 64:128.
            kT = big.tile([128, NFT, S], BF16, tag="kT")             # 4 MB
            qT = big.tile([128, NFT, TOK], BF16, tag="qT")           # 2 MB
            oT = big.tile([128, NFT, TOK], BF16, tag="oT")           # 2 MB
            wk = big.tile([128, NFT, D], BF16, tag="wk")             # 2 MB
            wv = big.tile([128, NFT, D], BF16, tag="wv")             # 2 MB
            wq = big.tile([128, NFT, D], BF16, tag="wq")             # 2 MB
            ident = big.tile([128, 128], BF16, tag="ident")

            def new_vaug():
                # [k-tile rows, kt, 8 heads x (dk | 1)] with ones in col dk
                va = vpool.tile([128, NKT, 8 * (DK + 1)], BF16, tag="vaug")
                nc.vector.memset(
                    va[:].rearrange("p t (h c) -> p t h c", h=8)[:, :, :, DK:DK + 1],
                    1.0,
                )
                return va

            vaug = [None, None]
            vaug[0] = new_vaug()

            def emit_scores_exp(hp, qg, kt):
                sc = ps_sc.tile([128, 1024], F32, tag="sc")
                nc.tensor.matmul(
                    sc[:, 0:512],
                    kT[0:64, hp, kt * 128:(kt + 1) * 128],
                    qT[0:64, hp, qg * 512:(qg + 1) * 512],
                    start=True, stop=True, tile_position=(0, 0),
                )
                nc.tensor.matmul(
                    sc[:, 512:1024],
                    kT[64:128, hp, kt * 128:(kt + 1) * 128],
                    qT[64:128, hp, qg * 512:(qg + 1) * 512],
                    start=True, stop=True, tile_position=(64, 0),
                )
                e_ab = ev.tile([128, 1024], BF16, tag="exp")
                nc.scalar.activation(
                    out=e_ab[:], in_=sc[:],
                    func=mybir.ActivationFunctionType.Exp,
                    scale=0.125,
                )
                return e_ab

            def emit_attnv(hg, j, kt, e_ab, o_psA, o_psB):
                for hh, o_ps in ((0, o_psA), (1, o_psB)):
                    va = vaug[hg][:, kt, :]
                    for qt in range(4):
                        # one accumulation group per psum bank: start marks
                        # the whole 2KB zero-region pending, so only the
                        # bank's first matmul starts and only its last
                        # stops; the other q-tiles' first writes land on
                        # pending-zero bytes (fresh write)
                        nc.tensor.matmul(
                            o_ps[:, qt * 128:qt * 128 + DK + 1],
                            e_ab[:, hh * 512 + qt * 128:
                                 hh * 512 + (qt + 1) * 128],
                            va[:, (2 * j + hh) * (DK + 1):
                               (2 * j + hh + 1) * (DK + 1)],
                            start=(kt == 0 and qt == 0),
                            stop=(kt == NKT - 1 and qt == 3),
                        )

            # x^T stays resident all kernel (loaded once), so projection
            # groups can be woven into any attention window in deadline
            # order instead of being tied to a streaming token-group sweep
            xall = big.tile([128, NFT, S], BF16, tag="xall")         # 4 MB

            # ---------- projection group emitters ----------
            # "k": (hgx, ft, tg)  K^T head-pair column block for one tg
            # "q": (hp, qgx)      Q^T head pair for one query group (tg==qg)
            # "v": (hgx, kt)      V row block -> v_aug[hgx]
            def groups_gen(worklist):
                for item in worklist:
                    kind = item[0]
                    ps = ps_mm.tile([128, 512], F32, tag="mm512")
                    if kind == "k":
                        _, hgx, ft, tg = item
                        for d in range(NFT):
                            nc.tensor.matmul(
                                ps[:],
                                wk[:, d, hgx * 512 + ft * 128:
                                   hgx * 512 + (ft + 1) * 128],
                                xall[:, d, tg * 512:(tg + 1) * 512],
                                start=(d == 0), stop=(d == NFT - 1),
                            )
                            yield
                        nc.vector.tensor_copy(
                            kT[:, hgx * 4 + ft, tg * 512:(tg + 1) * 512], ps[:])
                    elif kind == "q":
                        _, hp, qgx = item
                        for d in range(NFT):
                            nc.tensor.matmul(
                                ps[:], wq[:, d, hp * 128:(hp + 1) * 128],
                                xall[:, d, qgx * 512:(qgx + 1) * 512],
                                start=(d == 0), stop=(d == NFT - 1),
                            )
                            yield
                        nc.vector.tensor_copy(
                            qT[:, hp, qgx * 512:(qgx + 1) * 512], ps[:])
                    else:
                        _, hgx, kt = item
                        for d in range(NFT):
                            nc.tensor.matmul(
                                ps[:], xall[:, d, kt * 128:(kt + 1) * 128],
                                wv[:, d, hgx * 512:(hgx + 1) * 512],
                                start=(d == 0), stop=(d == NFT - 1),
                            )
                            yield
                        nc.vector.tensor_copy(
                            out=vaug[hgx][:, kt, :]
                            .rearrange("p (h c) -> p h c", h=8)[:, :, 0:DK],
                            in_=ps[:].rearrange("p (h c) -> p h c", h=8),
                        )

            def emit_groups(worklist):
                for _ in groups_gen(worklist):
                    pass

            # ---------- Phase A: minimal head before attention starts ----
            # B0's first two j-blocks need kT-ft0/ft1, Q-hp0 and all of V0;
            # every other projection is woven into the attention windows
            # DMA queue order = first-use order: x tg0 + the K columns the
            # head needs, then the remaining x groups ahead of the bulkier
            # weight halves (the head reads every token group early, but
            # only wk-lo/wv-lo/wq-hp0 before its last group)
            for d in range(NFT):
                # all four token groups of this d-tile together with the
                # head's K columns: the head's K-ft0 groups consume x
                # tg-major at ~1.7us per group, so every tg must land early
                nc.sync.dma_start(
                    xall[:, d, 0:512], xT_d.ap()[d * 128:(d + 1) * 128, 0:512])
                nc.sync.dma_start(wk[:, d, 0:256],
                                  wkT_d.ap()[d * 128:(d + 1) * 128, 0:256])
                nc.sync.dma_start(
                    xall[:, d, 512:1024],
                    xT_d.ap()[d * 128:(d + 1) * 128, 512:1024])
            for tg in range(2, NTG):
                for d in range(NFT):
                    nc.sync.dma_start(
                        xall[:, d, tg * 512:(tg + 1) * 512],
                        xT_d.ap()[d * 128:(d + 1) * 128, tg * 512:(tg + 1) * 512],
                    )
            for d in range(NFT):
                nc.sync.dma_start(wv[:, d, 0:512],
                                  wvT_d.ap()[d * 128:(d + 1) * 128, 0:512])
            for d in range(NFT):
                nc.sync.dma_start(wq[:, d, 0:128],
                                  wqT_d.ap()[d * 128:(d + 1) * 128, 0:128])
            for d in range(NFT):
                nc.sync.dma_start(wk[:, d, 256:1024],
                                  wkT_d.ap()[d * 128:(d + 1) * 128, 256:1024])
            for d in range(NFT):
                nc.sync.dma_start(wq[:, d, 128:1024],
                                  wqT_d.ap()[d * 128:(d + 1) * 128, 128:1024])
            nc.sync.dma_start(ident[:], ident_d.ap())
            for d in range(NFT):
                nc.sync.dma_start(wv[:, d, 512:1024],
                                  wvT_d.ap()[d * 128:(d + 1) * 128, 512:1024])

            vaug[1] = new_vaug()
            emit_groups([("k", 0, 0, tg) for tg in range(NTG)]
                        + [("k", 0, 1, tg) for tg in range(NTG)]
                        + [("q", 0, 0)]
                        + [("v", 0, kt) for kt in range(NKT)])

            # deadline-ordered weave lists. B0 consumes them at ~2.5/kt
            # (pull p lands near iteration p/2.5); B1-qg0 at 2/kt. Each
            # entry is one 8-matmul psum group.
            b0_list = ([("q", 1, 0)]                       # j1 queries
                       + [("k", 0, 2, tg) for tg in range(NTG)]   # j2 keys
                       + [("q", 2, 0)]
                       + [("k", 0, 3, tg) for tg in range(NTG)]   # j3 keys
                       + [("q", 3, 0)]
                       + [("q", hp, 1) for hp in range(4)]  # B0-qg1 queries
                       + [("k", 1, 0, tg) for tg in range(NTG)]   # B1-j0 keys
                       + [("q", 4, 0)]                      # B1-j0 queries
                       + [("v", 1, kt) for kt in range(NKT)]
                       + [("q", 5, 0)])
            b1_list = ([("k", 1, 1, tg) for tg in range(NTG)]
                       + [("k", 1, 2, 0), ("k", 1, 2, 1), ("q", 6, 0),
                          ("k", 1, 2, 2), ("k", 1, 2, 3), ("q", 7, 0)]
                       + [("k", 1, 3, tg) for tg in range(NTG)]
                       + [("q", hp, 1) for hp in range(4, 8)])

            woT_box = [None]

            # ---------- c_block: o-proj + residual + LayerNorm ----------
            if apply_bias:
                bo_bc = small.tile([128, D], F32, tag="bobc")
                nc.sync.dma_start(bo_bc[:],
                                  bo_d.ap()[0:1, :].broadcast_to((128, D)))
            if apply_gb:
                g_bc = small.tile([128, D], F32, tag="gbc")
                b_bc = small.tile([128, D], F32, tag="bbc")
                nc.sync.dma_start(g_bc[:],
                                  gb_d.ap()[0:1, :].broadcast_to((128, D)))
                nc.sync.dma_start(b_bc[:],
                                  gb_d.ap()[1:2, :].broadcast_to((128, D)))

            ys_tags = ("wv", "vaug", "xall")
            eps_t = small.tile([128, 1], F32, tag="eps")
            nc.vector.memset(eps_t[:], EPS)

            def c_block_gen(tt, alt=False):
                """o-proj (16 matmul pulls) then residual+LN+store.

                alt=True draws the psum from the scores pool — free once
                attention has ended — so consecutive tail blocks pipeline
                instead of serializing on ps_mm's two banks."""
                woT = woT_box[0]
                x_t = xr.tile([128, D], BF16, tag="xres")
                nc.sync.dma_start(x_t[:], xmy_d.ap()[tt * 128:(tt + 1) * 128, :])
                if alt:
                    big_ps = ps_sc.tile([128, 1024], F32, tag="sc")
                    pss = [big_ps[:, 0:512], big_ps[:, 512:1024]]
                else:
                    ps_e0 = ps_mm.tile([128, 512], F32, tag="mm512")
                    ps_e1 = ps_mm.tile([128, 512], F32, tag="mm512")
                    pss = [ps_e0[:], ps_e1[:]]
                for eh in range(2):
                    for ft in range(NFT):
                        nc.tensor.matmul(
                            pss[eh], oT[:, ft, tt * 128:(tt + 1) * 128],
                            woT[:, ft, eh * 512:(eh + 1) * 512],
                            start=(ft == 0), stop=(ft == NFT - 1),
                        )
                        yield
                ys_tag = ys_tags[tt % len(ys_tags)]
                if ys_tag == "vaug":
                    y_sb = vpool.tile([128, D], F32, tag=ys_tag)
                else:
                    y_sb = big.tile([128, D], F32, tag=ys_tag)
                for eh in range(2):
                    nc.vector.tensor_add(
                        y_sb[:, eh * 512:(eh + 1) * 512],
                        pss[eh], x_t[:, eh * 512:(eh + 1) * 512],
                    )
                if apply_bias:
                    nc.vector.tensor_add(y_sb[:], y_sb[:], bo_bc[:])
                stats = lnp.tile([128, 2, nc.vector.BN_STATS_DIM], F32, tag="st")
                nc.vector.bn_stats(stats[:, 0, :], y_sb[:, 0:512])
                nc.vector.bn_stats(stats[:, 1, :], y_sb[:, 512:1024])
                mv = lnp.tile([128, nc.vector.BN_AGGR_DIM], F32, tag="mv")
                nc.vector.bn_aggr(mv[:], stats[:])
                lnv = lnp.tile([128, 1], F32, tag="lnv")
                rstd = lnp.tile([128, 1], F32, tag="rstd")
                # rstd = exp(-0.5*ln(var+eps)): Ln and Exp share one ScalarE
                # activation table set, so the attention Exp table never
                # reloads mid-kernel (Sqrt's table does not contain Exp)
                nc.scalar.activation(
                    out=lnv[:], in_=mv[:, 1:2],
                    func=mybir.ActivationFunctionType.Ln,
                    bias=eps_t[:], scale=1.0,
                )
                nc.scalar.activation(
                    out=rstd[:], in_=lnv[:],
                    func=mybir.ActivationFunctionType.Exp,
                    scale=-0.5,
                )
                nc.vector.tensor_scalar(
                    out=y_sb[:], in0=y_sb[:],
                    scalar1=mv[:, 0:1], scalar2=rstd[:],
                    op0=mybir.AluOpType.subtract, op1=mybir.AluOpType.mult,
                )
                if apply_gb:
                    nc.vector.tensor_mul(y_sb[:], y_sb[:], g_bc[:])
                    nc.vector.tensor_add(y_sb[:], y_sb[:], b_bc[:])
                nc.sync.dma_start(y_d.ap()[tt * 128:(tt + 1) * 128, :], y_sb[:])

            def pull(gen, n):
                if gen is None:
                    return None
                for _ in range(n):
                    try:
                        next(gen)
                    except StopIteration:
                        return None
                return gen

            def drain(gen):
                if gen is not None:
                    for _ in gen:
                        pass

            # ---------- Phase B: attention (hg0 then hg1) ----------
            blocks = [(hg, qg, j)
                      for hg in range(2) for qg in range(2) for j in range(4)]
            weave = groups_gen(b0_list)
            weave_n = (3, 2)        # ~2.5 pulls/kt through B0
            lead = {}
            for bi, (hg, qg, j) in enumerate(blocks):
                hp = hg * 4 + j
                if bi == 8:
                    # B1-qg0: late projection groups fill its ScalarE-bound
                    # slack (K1-ft1..3 land one j-block ahead of their use)
                    weave = groups_gen(b1_list)
                    weave_n = (2, 2)
                elif bi >= 12:
                    # c_block for the query group finished one step ago:
                    # tts 0-3 here, the tail covers 4-7
                    weave = c_block_gen(qg * 4 + j - 4)
                    weave_n = (1, 1)
                o_psA = ps_o.tile([128, 512], F32, tag="o")
                o_psB = ps_o.tile([128, 512], F32, tag="o")
                lead_emitted = bi + 1 >= len(blocks)
                for kt in range(NKT):
                    e_ab = lead.pop((hp, qg, kt), None)
                    if e_ab is None:
                        e_ab = emit_scores_exp(hp, qg, kt)
                    if kt == NKT - 1 and not lead_emitted:
                        # pre-emit the next block's first two scores+exps
                        # BEFORE this attnV: attnV(kt15) waits on exp(kt15)
                        # and its 8 matmuls overflow PE's 4-deep wait queue,
                        # so anything after it stalls; emitting the lead
                        # scores first keeps ScalarE fed across the boundary
                        nhg, nqg, nj = blocks[bi + 1]
                        for ktl in range(2):
                            lead[(nhg * 4 + nj, nqg, ktl)] = \
                                emit_scores_exp(nhg * 4 + nj, nqg, ktl)
                        lead_emitted = True
                    emit_attnv(hg, j, kt, e_ab, o_psA, o_psB)
                    if kt == NKT - 1 and lead_emitted and bi + 1 < len(blocks):
                        # two more leads right after the last attnV: four
                        # queued exps (~4.2us) keep ScalarE busy through the
                        # epilogue chain (normalize -> transpose -> evict ->
                        # next block's o_ps rotation, ~3.4us) that delays
                        # the next block's own scores
                        nhg, nqg, nj = blocks[bi + 1]
                        for ktl in (2, 3):
                            if (nhg * 4 + nj, nqg, ktl) not in lead:
                                lead[(nhg * 4 + nj, nqg, ktl)] = \
                                    emit_scores_exp(nhg * 4 + nj, nqg, ktl)
                    weave = pull(weave, weave_n[kt % 2])
                if not lead_emitted:
                    nhg, nqg, nj = blocks[bi + 1]
                    for ktl in range(2):
                        lead[(nhg * 4 + nj, nqg, ktl)] = \
                            emit_scores_exp(nhg * 4 + nj, nqg, ktl)
                if bi == 7 or bi >= 11:
                    # end of B0 / end of B1-qg0: finish weave leftovers;
                    # c_blocks: run the woven block's finalize
                    drain(weave)
                    weave = None
                if bi == 11:
                    # wk's readers (b1_list K groups) are all emitted: its
                    # slot now takes the o-proj weights, first-used half
                    # first; the c_blocks start ~15us later
                    woT_t = big.tile([128, NFT, D], BF16, tag="wk")
                    for d in range(NFT):
                        nc.sync.dma_start(
                            woT_t[:, d, 0:512],
                            woT_d.ap()[d * 128:(d + 1) * 128, 0:512])
                    for d in range(NFT):
                        nc.sync.dma_start(
                            woT_t[:, d, 512:1024],
                            woT_d.ap()[d * 128:(d + 1) * 128, 512:1024])
                    woT_box[0] = woT_t
                # normalize by the softmax sums (psum col 64 of each
                # q-tile), then PE-transpose into o-proj layout
                for hh, o_ps in ((0, o_psA), (1, o_psB)):
                    rec = onp.tile([128, 4], F32, tag="rec")
                    nc.vector.reciprocal(
                        rec[:],
                        o_ps[:].rearrange("p (q c) -> p q c", c=128)[:, :, DK],
                    )
                    o_nrm = onp.tile([128, 4, DK], BF16, tag="onrm")
                    for qt in range(4):
                        nc.vector.tensor_scalar(
                            out=o_nrm[:, qt, :],
                            in0=o_ps[:, qt * 128:qt * 128 + DK],
                            scalar1=rec[:, qt:qt + 1], scalar2=None,
                            op0=mybir.AluOpType.mult,
                        )
                    tr = ps_o.tile([128, 512], BF16, tag="o")
                    for qt in range(4):
                        nc.tensor.transpose(
                            tr[0:DK, qt * 128:(qt + 1) * 128],
                            o_nrm[:, qt, :], ident[:],
                        )
                    nc.vector.tensor_copy(
                        oT[hh * 64:(hh + 1) * 64, hp,
                           qg * 512:(qg + 1) * 512],
                        tr[0:DK, :],
                    )

            # tail: c_blocks for the last query group
            for tt in range(4, 8):
                drain(c_block_gen(tt, alt=bool(tt % 2)))

    nc.compile()
    return nc


def kernel(x, w_q, w_k, w_v, w_o, b_o, ln_g, ln_b):
    import ml_dtypes

    x = np.asarray(x, dtype=np.float32)
    w_q = np.asarray(w_q, dtype=np.float32)
    w_k = np.asarray(w_k, dtype=np.float32)
    w_v = np.asarray(w_v, dtype=np.float32)
    w_o = np.asarray(w_o, dtype=np.float32)
    b_o = np.asarray(b_o, dtype=np.float32)
    ln_g = np.asarray(ln_g, dtype=np.float32)
    ln_b = np.asarray(ln_b, dtype=np.float32)

    apply_gb = not (np.all(ln_g == 1.0) and np.all(ln_b == 0.0))
    apply_bias = bool(np.any(b_o != 0.0))
    key = (apply_gb, apply_bias)
    if key not in _CACHE:
        _CACHE[key] = build(apply_gb, apply_bias)
    nc = _CACHE[key]

    bf16 = ml_dtypes.bfloat16
    wqT = np.ascontiguousarray(w_q.T).astype(bf16)
    wkT = np.ascontiguousarray(w_k.T).astype(bf16)
    wvT = np.ascontiguousarray(w_v.T).astype(bf16)
    woT = np.ascontiguousarray(w_o.T).astype(bf16)
    ident = np.eye(128, dtype=np.float32).astype(bf16)
    gb = np.stack([ln_g, ln_b]).astype(np.float32)
    bo = np.ascontiguousarray(b_o.reshape(1, D))

    in_maps = []
    for c in range(N_CORES):
        b = c // 2
        half = c % 2
        xb = x[b]
        xT = np.ascontiguousarray(xb.T)
        if half == 1:
            xT = np.ascontiguousarray(np.roll(xT, -TOK, axis=1))
        xmy = np.ascontiguousarray(xb[half * TOK:(half + 1) * TOK]).astype(bf16)
        in_maps.append({
            "xT": xT.astype(bf16), "xmy": xmy,
            "wqT": wqT, "wkT": wkT, "wvT": wvT, "woT": woT,
            "ident": ident, "bo": bo, "gb": gb,
        })

    res = bass_utils.run_bass_kernel_spmd(nc, in_maps, core_ids=list(range(N_CORES)))
    y = np.stack([res.results[c]["y"] for c in range(N_CORES)])
    return y.reshape(B, S, D)
